# revision 24
# baseline (speedup 1.0000x reference)
"""Trainium2 Bass kernel for nn_DiscreteSequenceModel (GRU rollout).

Math (see reference): h0 = y0 @ enc_w.T + enc_b, then T=512 sequential GRU
steps with input == hidden (ts values are unused by the math; only len(ts)
matters), emitting pred_t = h_t @ dec_w.T + dec_b before each update.

Sharding: data-parallel over batch B=256 across 8 cores (32 rows/core),
weights replicated.  Per core, each step contracts h (K=1024, 8 k-tiles of
128) against a packed weight matrix with 4352 output columns
(r|z|hn|pred|inn per feature-group).  To keep the 128x128 PE array full
with only 32 batch rows, we column-tile the array into 4 groups of 32:
every group holds the same stationary h-tile but streams a different
feature-chunk of the weights, so the array does 4 concurrent 32-wide
matmuls (128 effective rows).  Gate outputs land in PSUM as
[4 groups x 32 batch, features], which is also the layout the elementwise
GRU math wants (128 full partitions).  The next step's stationary h^T
tiles are produced with two PE transposes of the new hidden state.

Biases (gate bias, bias_n, dec_b) are folded into the contraction as a
9th K=1 "tile" (ones vector x bias row).  Matmuls run in fp16 (the PE's 4-byte float path rejects column tiling;
fp16 keeps a tf32-grade 10-bit mantissa and our values are small), PSUM
accumulation in fp32, and the carried hidden state stays fp32 — only the
matmul operands round.
"""

import numpy as np

import concourse.bacc as bacc
import concourse.bass as bass
import concourse.tile as tile
from concourse import mybir
from concourse.bass_utils import run_bass_kernel_spmd

B, T, H, D = 256, 512, 1024, 256
NCORES = 8
BL = B // NCORES            # 32 batch rows per core
NG = 4                      # PE column-tile groups
FC = H // NG                # 256 gate features per group
PC = D // NG                # 64 decoder features per group
GW = 4 * FC + PC            # 1088 packed weight cols per group
KT = H // 128               # 8 k-tiles

# free-dim offsets inside one group's 1088-wide strip
RZ0, RZ1 = 0, 2 * FC             # [r 256 | z 256]
HN0, HN1 = 2 * FC, 3 * FC        # [hn 256]
PI0, PI1 = 3 * FC, GW            # [pred 64 | inn 256]

# blob column layout (one [128, BLOBW] fp32 tensor holds every constant)
OFF_WS = 0                       # weights: [p, k*4352 + j*1088 + c]
OFF_ES = OFF_WS + KT * NG * GW   # encoder weights: [p, k*1024 + j*256 + c]
OFF_Y0T = OFF_ES + 2 * H         # y0^T k-tiles: [p, k*32 + b]
OFF_BR = OFF_Y0T + 2 * BL        # row 0: packed gate/pred bias row (4352)
OFF_EB = OFF_BR + NG * GW        # row 0: packed encoder bias row (1024)
OFF_ID = OFF_EB + H              # 128x128 identity (for PE transpose)
OFF_ONES = OFF_ID + 128          # row 0: 32 ones (bias k-tile stationary)
BLOBW = OFF_ONES + BL

F32 = mybir.dt.float32
FP16 = mybir.dt.float16
AFT = mybir.ActivationFunctionType


def _emit(tc, nc, blob, idf, preds, steps, unroll, clobber=False):
    assert steps % unroll == 0 and unroll % 2 == 0
    import contextlib

    with contextlib.ExitStack() as ctx:
        const = ctx.enter_context(tc.tile_pool(name="const", bufs=1))
        C = const.tile([128, BLOBW], FP16)
        # h state as separate L/R half-tiles so the elementwise tail can
        # release each half to the PE transposes independently
        HL = [const.tile([128, 128], F32, name="HL0"),
              const.tile([128, 128], F32, name="HL1")]
        HR = [const.tile([128, 128], F32, name="HR0"),
              const.tile([128, 128], F32, name="HR1")]

        IDT = const.tile([128, 128], F32)
        nc.sync.dma_start(C[:], blob[:])
        nc.sync.dma_start(IDT[:], idf[:])

        def ws(k, j, c0, c1):
            o = OFF_WS + k * NG * GW + j * GW
            return C[:, o + c0:o + c1]

        ID = IDT[:]
        ONES = C[0:1, OFF_ONES:OFF_ONES + BL]

        ps = ctx.enter_context(tc.tile_pool(name="ps", bufs=1, space="PSUM"))
        ps_rz = ps.tile([128, 2 * FC], F32)
        ps_hn = ps.tile([128, FC], F32)
        ps_pi = ps.tile([128, PC + FC], F32)
        ps_t1 = ps.tile([128, 128], F32)
        ps_t2 = ps.tile([128, 128], F32)

        sb = ctx.enter_context(tc.tile_pool(name="sb", bufs=2))

        # ---- encoder: h0 in gate layout [32j+b, f_local] ----
        for j in range(NG):
            for k in range(2):
                nc.tensor.matmul(
                    ps_rz[32 * j:32 * j + 32, 0:FC],
                    C[:, OFF_Y0T + k * BL:OFF_Y0T + (k + 1) * BL],
                    C[:, OFF_ES + k * H + j * FC:
                      OFF_ES + k * H + (j + 1) * FC],
                    start=(k == 0), stop=False, skip_group_check=True,
                    tile_position=(0, 32 * j))
            nc.tensor.matmul(
                ps_rz[32 * j:32 * j + 32, 0:FC],
                ONES,
                C[0:1, OFF_EB + j * FC:OFF_EB + (j + 1) * FC],
                start=False, stop=True, skip_group_check=True,
                tile_position=(0, 32 * j))
        nc.scalar.copy(HL[0][:], ps_rz[:, 0:128])
        nc.vector.tensor_copy(HR[0][:], ps_rz[:, 128:256])

        def step_body(tv, sub, stage):
            parity = sub % 2
            hcl, hcr = HL[parity][:], HR[parity][:]
            hnl, hnr = HL[1 - parity][:], HR[1 - parity][:]

            hTe = sb.tile([128, 128], FP16, tag="hTe")
            hTo = sb.tile([128, 128], FP16, tag="hTo")

            def mm(out_ap, k, j, c0, c1, start, stop):
                if k < KT:
                    src = hTe if k % 2 == 0 else hTo
                    m = k // 2
                    lhsT = src[:, m * 32:(m + 1) * 32]
                    rhs = ws(k, j, c0, c1)
                else:  # bias "k-tile": ones x bias row
                    lhsT = ONES
                    rhs = C[0:1, OFF_BR + j * GW + c0:OFF_BR + j * GW + c1]
                nc.tensor.matmul(out_ap, lhsT, rhs,
                                 start=start, stop=stop,
                                 skip_group_check=True,
                                 tile_position=(0, 32 * j))

            # bias "k-tiles" first: they depend only on constants, so the PE
            # runs them while the previous sub-step's elementwise tail is
            # still producing h (fills the PE's dependency bubble).
            for j in range(NG):
                mm(ps_rz[32 * j:32 * j + 32, :], KT, j, RZ0, RZ1, True, False)
            for j in range(NG):
                mm(ps_hn[32 * j:32 * j + 32, :], KT, j, HN0, HN1, True, False)

            # left-half transpose as soon as h_left lands, then the even
            # k-tiles (which live in hTe) while the right half finishes
            nc.tensor.transpose(ps_t1[:], hcl, ID)
            nc.scalar.copy(hTe[:], ps_t1[:])
            for k in range(0, KT, 2):
                for j in range(NG):
                    mm(ps_rz[32 * j:32 * j + 32, :], k, j, RZ0, RZ1,
                       False, False)
            nc.tensor.transpose(ps_t2[:], hcr, ID)
            nc.vector.tensor_copy(hTo[:], ps_t2[:])
            for j in range(NG):
                mm(ps_pi[32 * j:32 * j + 32, :], KT, j, PI0, PI1, True, False)
            for k in range(1, KT, 2):
                for j in range(NG):
                    mm(ps_rz[32 * j:32 * j + 32, :], k, j, RZ0, RZ1,
                       False, k == KT - 1)

            r = sb.tile([128, FC], F32, tag="r")
            z = sb.tile([128, FC], F32, tag="z")
            omz = sb.tile([128, FC], F32, tag="omz")
            zh = sb.tile([128, FC], F32, tag="zh")
            nc.scalar.activation(r[:], ps_rz[:, 0:FC], AFT.Sigmoid)
            nc.scalar.activation(z[:], ps_rz[:, FC:2 * FC], AFT.Sigmoid)
            # 1 - sigmoid(x) == sigmoid(-x)
            nc.scalar.activation(omz[:], ps_rz[:, FC:2 * FC], AFT.Sigmoid,
                                 scale=-1.0)
            nc.vector.tensor_mul(zh[:, 0:128], z[:, 0:128], hcl)
            nc.vector.tensor_mul(zh[:, 128:256], z[:, 128:256], hcr)

            # pass B1: hn — finishes early so v = r*hn hides under pass B2
            for k in range(KT):
                for j in range(NG):
                    mm(ps_hn[32 * j:32 * j + 32, :], k, j, HN0, HN1,
                       False, k == KT - 1)

            v = sb.tile([128, FC], F32, tag="v")
            nc.vector.tensor_mul(v[:], r[:], ps_hn[:])

            # pass B2: pred|inn
            for k in range(KT):
                for j in range(NG):
                    mm(ps_pi[32 * j:32 * j + 32, :], k, j, PI0, PI1,
                       False, k == KT - 1)

            nc.scalar.copy(stage[:, sub * PC:(sub + 1) * PC],
                           ps_pi[:, 0:PC])

            # tail in L/R halves: the left half reaches h' (and the next
            # sub-step's T1) while the right half is still in flight
            w2l = sb.tile([128, 128], F32, tag="w2l")
            w2r = sb.tile([128, 128], F32, tag="w2r")
            ntl = sb.tile([128, 128], F32, tag="ntl")
            ntr = sb.tile([128, 128], F32, tag="ntr")
            t4l = sb.tile([128, 128], F32, tag="t4l")
            t4r = sb.tile([128, 128], F32, tag="t4r")
            nc.vector.tensor_add(w2l[:], v[:, 0:128], ps_pi[:, PC:PC + 128])
            nc.vector.tensor_add(w2r[:], v[:, 128:256],
                                 ps_pi[:, PC + 128:PC + 256])
            nc.scalar.activation(ntl[:], w2l[:], AFT.Tanh)
            nc.scalar.activation(ntr[:], w2r[:], AFT.Tanh)
            nc.vector.tensor_mul(t4l[:], ntl[:], omz[:, 0:128])
            nc.vector.tensor_add(hnl, t4l[:], zh[:, 0:128])
            nc.vector.tensor_mul(t4r[:], ntr[:], omz[:, 128:256])
            nc.vector.tensor_add(hnr, t4r[:], zh[:, 128:256])

        # Raw pred layout: preds_raw[32j+b, t*64+c] = pred[b, t, 64j+c];
        # the host untangles (j,b) afterwards.  One 2D DMA per body keeps
        # the loop at a single HW-DGE queue (the back-edge drain and the
        # PE's LDWEIGHTS descriptor only support a few sync waits).
        with tc.For_i(0, steps, unroll,
                      hint_engines=(mybir.EngineType.PE,)) as iv:
            stage = sb.tile([128, unroll * PC], F32, tag="predstage")
            for s in range(unroll):
                step_body(iv + s, s, stage)
            if clobber:  # timing-only build: fixed dst slot
                nc.sync.dma_start(preds[:, 0:unroll * PC], stage[:])
            else:
                nc.sync.dma_start(preds[:, bass.ds(iv * PC, unroll * PC)],
                                  stage[:])


_CACHE = {}


def _get_nc(steps, unroll, clobber=False, out_steps=None):
    key = (steps, unroll, clobber)
    if key in _CACHE:
        return _CACHE[key]
    nc = bacc.Bacc("TRN2", target_bir_lowering=False, debug=False,
                   enable_asserts=False, num_devices=NCORES)
    blob = nc.dram_tensor("blob", [128, BLOBW], FP16,
                          kind="ExternalInput").ap()
    idf = nc.dram_tensor("idf", [128, 128], F32, kind="ExternalInput").ap()
    preds = nc.dram_tensor("preds", [128, (out_steps or steps) * PC], F32,
                           kind="ExternalOutput").ap()
    with tile.TileContext(nc) as tc:
        _emit(tc, nc, blob, idf, preds, steps, unroll, clobber=clobber)
    nc.compile()
    _CACHE[key] = nc
    return nc


def _pack(y0_batch, enc_w, enc_b, w_ih, w_hh, bias, bias_n, dec_w, dec_b):
    f = lambda x: np.ascontiguousarray(np.asarray(x, dtype=np.float32))
    y0_batch, enc_w, enc_b = f(y0_batch), f(enc_w), f(enc_b)
    w_ih, w_hh, bias, bias_n = f(w_ih), f(w_hh), f(bias), f(bias_n)
    dec_w, dec_b = f(dec_w), f(dec_b)

    W_r = w_ih[0:H] + w_hh[0:H]
    W_z = w_ih[H:2 * H] + w_hh[H:2 * H]
    W_ni = w_ih[2 * H:3 * H]
    W_nh = w_hh[2 * H:3 * H]

    wcols, bcols = [], []
    for j in range(NG):
        f0, f1 = j * FC, (j + 1) * FC
        p0, p1 = j * PC, (j + 1) * PC
        wcols += [W_r[f0:f1].T, W_z[f0:f1].T, W_nh[f0:f1].T,
                  dec_w[p0:p1].T, W_ni[f0:f1].T]
        bcols += [bias[f0:f1], bias[H + f0:H + f1], bias_n[f0:f1],
                  dec_b[p0:p1], bias[2 * H + f0:2 * H + f1]]

    base = np.zeros((128, BLOBW), np.float32)
    wbig = np.concatenate(wcols, axis=1)            # [1024, 4352]
    base[:, OFF_WS:OFF_ES] = (
        wbig.reshape(KT, 128, NG * GW).transpose(1, 0, 2).reshape(128, -1))
    ebig = np.concatenate(
        [enc_w[j * FC:(j + 1) * FC, :].T for j in range(NG)], axis=1)
    base[:, OFF_ES:OFF_Y0T] = (
        ebig.reshape(2, 128, H).transpose(1, 0, 2).reshape(128, -1))
    base[0, OFF_BR:OFF_EB] = np.concatenate(bcols)
    base[0, OFF_EB:OFF_ID] = np.concatenate(
        [enc_b[j * FC:(j + 1) * FC] for j in range(NG)])
    base[:, OFF_ID:OFF_ONES] = np.eye(128, dtype=np.float32)
    base[0, OFF_ONES:BLOBW] = 1.0

    idf = np.ascontiguousarray(np.eye(128, dtype=np.float32))
    in_maps = []
    for c in range(NCORES):
        bc = base.copy()
        y0t = y0_batch[c * BL:(c + 1) * BL].T       # [256, 32]
        bc[:, OFF_Y0T:OFF_BR] = (
            y0t.reshape(2, 128, BL).transpose(1, 0, 2).reshape(128, -1))
        in_maps.append(dict(blob=bc.astype(np.float16), idf=idf))
    return in_maps


def _pick_unroll(steps):
    for u in (16, 8, 4, 2):
        if steps % u == 0:
            return u
    return 1


def _run(inputs, steps=T, unroll=None, **run_kwargs):
    if unroll is None:
        unroll = _pick_unroll(steps)
    in_maps = _pack(
        inputs["y0_batch"], inputs["enc_w"], inputs["enc_b"], inputs["w_ih"],
        inputs["w_hh"], inputs["bias"], inputs["bias_n"], inputs["dec_w"],
        inputs["dec_b"])
    nc = _get_nc(steps, unroll)
    res = run_bass_kernel_spmd(nc, in_maps, core_ids=list(range(NCORES)),
                               **run_kwargs)
    # preds_raw[32j+b, t*64+c] -> [b, t, 64j+c]
    outs = []
    for r in res.results:
        raw = r["preds"].reshape(NG, BL, steps, PC)
        outs.append(np.ascontiguousarray(raw.transpose(1, 2, 0, 3))
                    .reshape(BL, steps, D))
    return np.concatenate(outs, axis=0), res


def kernel(ts=None, y0_batch=None, enc_w=None, enc_b=None, w_ih=None,
           w_hh=None, bias=None, bias_n=None, dec_w=None, dec_b=None):
    steps = int(np.asarray(ts).shape[0]) if ts is not None else T
    out, _ = _run(dict(y0_batch=y0_batch, enc_w=enc_w, enc_b=enc_b,
                       w_ih=w_ih, w_hh=w_hh, bias=bias, bias_n=bias_n,
                       dec_w=dec_w, dec_b=dec_b), steps=steps)
    return out
